# revision 54
# baseline (speedup 1.0000x reference)
"""Trainium2 Bass kernel for nn_BidirectionalTemporalAttention.

Reference computation (B=2, T=16, F=128, D=1024, N=T*F=2048):
  xf = x.reshape(B, N, D)
  lookback branch: 8 heads, E=64, causal mask (keep k <= q)
  lookahead branch: 8 heads, anti-causal (keep k >= q)
  o = concat([o_lb, o_la], heads) -> (B, 16, N, 64) -> RAW reshape (B, N, D)
  out = o @ Wo^T -> (B, T, F, D)

The raw reshape means out row r = h*128 + g depends only on head h (tokens
16g..16g+15 of that head).  So with 4 heads per core each core's 512 output
rows are fully local: no collectives, the host just concatenates row slices.

Sharding over 8 cores: (batch b in 2) x (group in [lb0-3, lb4-7, la0-3, la4-7]).
Lookahead cores receive the token-reversed sequence so one SPMD causal program
serves all cores; their outputs are un-reversed on the host (row reversal
within each 128-row head block, plus a j-group reversal folded into Wo).

Per-core kernel layout choices:
  - all matmul operands in bf16 (same 1 cycle/row PE speed as float32r, but
    no F>=256 cliff, half the DMA/SBUF traffic, 2-4x DVE elementwise speed);
    PSUM accumulation stays fp32, final output fp32.
  - S^T blocks [k(128 part), q(512 free)] so softmax-denominator and PV both
    contract over k on the partition axis.
  - causal structure exploited at 128-column granularity: for the diagonal
    key-block dg the S matmul / exp / PV only cover q-columns [128*dg, 512)
    (the rest is fully masked), and only the 128-wide triangle band gets a
    0/1 mask multiply.  PV accumulates partial column ranges into PSUM
    (skip_group_check: hardware is range-exact, only the sim's zero-region
    bookkeeping would object).
  - exp has no max-subtraction (scores are O(10), safe in fp32); softmax
    denominator comes free as a ones-column appended to V in the PV matmul.
  - PSUM->SBUF bounces for Q/K/V run on the Scalar engine so the Vector
    queue (mask multiplies, normalize) never gates the next chunk's
    projection matmuls; chunk c+1's projections are emitted between
    attention(0,c) and attention(1,c) so their copies land a block early.
  - softmax normalization (reciprocal/broadcast/scale) is deferred and
    dripped as single ops into later idle vector-queue slots so the 3.3us
    DVE reciprocal never queues ahead of mask multiplies that gate PV.
    The final block instead normalizes straight from PSUM with a scalar
    exp(-ln(den)) (scalar queue is idle by then; same activation table).
  - attention output written into O2T [128=(n%2)*64+e, n//2]; its strided
    views O2T[:, m::8] are exactly the K=128 lhsT tiles the out-projection
    needs under the reference's raw (H,N,E)->(N,D) reshape.  Three of the
    pr=0 out-projection groups are interleaved into the last attention
    block's kb loop: the in-order PE queue cannot slide ready work past a
    stalled PV on its own, and the scalar exp stream (~1.1us/key-block)
    otherwise starves the PE there.
  - emission-order rules learned on HW: never defer PV matmuls relative to
    their S/exp/mask (three reordering variants all regressed); keep PSUM
    bounces on scalar, masks + normalize drips on vector.
"""

import sys

if "/opt/trn_rl_repo" not in sys.path:
    sys.path.insert(0, "/opt/trn_rl_repo")

import ml_dtypes
import numpy as np

import concourse.bass as bass  # noqa: F401
import concourse.mybir as mybir
import concourse.tile as tile
from concourse import bacc
from concourse.bass_utils import run_bass_kernel_spmd

F32 = mybir.dt.float32
BF16 = mybir.dt.bfloat16
EXP = mybir.ActivationFunctionType.Exp
BF16_NP = ml_dtypes.bfloat16

N = 2048  # tokens per batch
D = 1024  # embed dim
E = 64  # head dim
HPC = 4  # heads per core
NQB = 4  # query blocks of 512
NKB = 16  # key blocks of 128
NDB = 8  # d blocks of 128
SCALE = 0.125  # 1/sqrt(E)

_CACHE = {}


def build_nc():
    nc = bacc.Bacc("TRN2", target_bir_lowering=False, debug=False)

    F32R = mybir.dt.float32r
    xt_d = nc.dram_tensor("xt", [D, N], BF16, kind="ExternalInput").ap()
    wq_d = nc.dram_tensor("wq", [128, NDB, 256], BF16, kind="ExternalInput").ap()
    wk_d = nc.dram_tensor("wk", [128, NDB, 256], BF16, kind="ExternalInput").ap()
    wv_d = nc.dram_tensor("wv", [128, NDB, 256], BF16, kind="ExternalInput").ap()
    wo_d = nc.dram_tensor("wo", [128, 8, D], F32R, kind="ExternalInput").ap()
    mk_d = nc.dram_tensor("mk", [128, 2, 128], BF16, kind="ExternalInput").ap()
    out_d = nc.dram_tensor("out", [512, D], F32, kind="ExternalOutput").ap()

    with tile.TileContext(nc) as tc:
        with (
            tc.tile_pool(name="w", bufs=1) as wp,
            tc.tile_pool(name="xp", bufs=3) as xp,
            tc.tile_pool(name="qkv", bufs=1) as qkvp,
            tc.tile_pool(name="pt", bufs=3) as ptp,
            tc.tile_pool(name="o2t", bufs=4) as o2tp,
            tc.tile_pool(name="osb", bufs=6) as osbp,
            tc.tile_pool(name="ob", bufs=2) as obp,
            tc.tile_pool(name="rc", bufs=2) as rcp,
            tc.tile_pool(name="rc1", bufs=2) as rcp1,
            tc.tile_pool(name="pss", bufs=2, space="PSUM") as pss,
            tc.tile_pool(name="psq", bufs=2, space="PSUM") as psq,
        ):
            # --- weights / constants (wq + x first: they gate the first MMs;
            #     wo/mk are deferred behind them so they don't queue ahead) ---
            wq_sb = wp.tile([128, NDB, 256], BF16, tag="wq")
            wk_sb = wp.tile([128, NDB, 256], BF16, tag="wk")
            wv_sb = wp.tile([128, NDB, 256], BF16, tag="wv")
            mk_sb = wp.tile([128, 2, 128], BF16, tag="mk")
            xt_r = xt_d.rearrange("(o p) n -> p o n", p=128)
            # first Q-proj matmul needs only (wq do0, x do0): land those two
            # small transfers first so the PE starts ~5us earlier
            nc.sync.dma_start(wq_sb[:, 0:1, :], wq_d[:, 0:1, :])
            xc0 = xp.tile([128, NDB, 512], BF16, tag="xc", name="xc_0")
            nc.sync.dma_start(xc0[:, 0:1, :], xt_r[:, 0:1, 0:512])
            nc.sync.dma_start(wq_sb[:, 1:8, :], wq_d[:, 1:8, :])
            nc.sync.dma_start(xc0[:, 1:4, :], xt_r[:, 1:4, 0:512])
            nc.sync.dma_start(xc0[:, 4:8, :], xt_r[:, 4:8, 0:512])
            nc.sync.dma_start(wk_sb[:], wk_d)
            nc.sync.dma_start(wv_sb[:], wv_d)
            nc.sync.dma_start(mk_sb[:], mk_d)
            wo_parts = []
            for i, tg in ((0, "wo2a"), (1, "wo2b")):
                wpart = wp.tile([128, 2, D], F32R, tag=tg, name=f"wo_{i}")
                nc.sync.dma_start(wpart[:], wo_d[:, 2 * i : 2 * i + 2, :])
                wo_parts.append(wpart)
            ones_bf = wp.tile([128, 64], BF16, tag="ones_bf")
            nc.vector.memset(ones_bf[:], 1.0)

            # persistent Q^T / K^T / V(+ones column; padded to 68 so each
            # [128, 68] lhsT slice starts 4-byte aligned)
            qt = qkvp.tile([128, 2, N], BF16, tag="qt")  # [(2h)*64e, pair, n]
            kt = qkvp.tile([128, 2, N], BF16, tag="kt")
            vt = qkvp.tile([128, NKB, HPC, 68], BF16, tag="vt")  # [k, kb, h, e|1|pad]
            nc.vector.memset(vt[:, :, :, 65:68], 0.0)
            nc.vector.tensor_copy(
                vt[:, :, :, 64],
                ones_bf[:, 0 : NKB * HPC].rearrange("p (a b) -> p a b", a=NKB),
            )

            o2t_all = {
                pr: [
                    o2tp.tile([128, N // 2], F32R, tag="o2", name=f"o2t_{pr}_{i}")
                    for i in range(2)
                ]
                for pr in range(2)
            }
            tails = [[]]  # deferred normalize work from previous blocks

            def drip(n=1):
                # emit up to n queued normalize ops; callers place these in
                # spots where the vector queue is otherwise idle
                for _ in range(n):
                    if not tails[0]:
                        return
                    tails[0].pop(0)()

            def make_normalize_tail(pr, qb, osb_h, den2, mul_eng=None):
                # 1/den + per-head partition broadcast + column-scale mults.
                # Returned as closures that attention() drips into the next
                # block's kb loop, so the 4us DVE reciprocal never queues
                # ahead of the mask mults that gate PV matmuls.
                # NB reciprocal_approx_fast returns garbage on HW here
                # (custom-DVE table issue) — plain reciprocal only.
                rec2 = rcp1.tile([33, 512], F32, tag="rec2", name=f"rc2_{pr}_{qb}")
                rec_b = rcp1.tile([1, 512], F32, tag="rec_b", name=f"rcb_{pr}_{qb}")
                brs = [
                    rcp1.tile([64, 512], F32, tag=f"brs{h}", name=f"brs_{pr}_{qb}_{h}")
                    for h in range(2)
                ]
                mul_eng = mul_eng or nc.vector

                def mult(h, par):
                    return mul_eng.tensor_mul(
                        o2t_all[pr][h][
                            64 * par : 64 * par + 64,
                            256 * qb : 256 * (qb + 1),
                        ],
                        osb_h[h][0:64, par::2],
                        brs[h][:, par::2],
                    )

                # both broadcasts issue right after their sources so no
                # mult ever waits on gpsimd inside the vector FIFO (the
                # mults are what the next block's mask mults queue behind)
                return [
                    lambda: nc.vector.reciprocal(rec2[:], den2[:]),
                    lambda: nc.gpsimd.partition_broadcast(brs[0][:], rec2[0:1, :]),
                    lambda: nc.vector.tensor_copy(rec_b[:], rec2[32:33, :]),
                    lambda: nc.gpsimd.partition_broadcast(brs[1][:], rec_b[:]),
                    lambda: mult(0, 0),
                    lambda: mult(0, 1),
                    lambda: mult(1, 0),
                    lambda: mult(1, 1),
                ]

            def attention(pr, qb, last=False, extra=None):
                nkb = 4 * qb + 4  # kept key blocks (causal)
                o_ps = [
                    pss.tile([128, 512], F32, tag="ov", name=f"ov_{pr}_{qb}_{i}")
                    for i in range(2)
                ]
                def emit_pv(kb, pt_t, lo):
                    for h in range(2):
                        nc.tensor.matmul(
                            o_ps[h][0:68, lo:512],
                            vt[:, kb, 2 * pr + h, :],
                            pt_t[:, h, lo:512],
                            start=(kb == 0),
                            stop=(kb == nkb - 1),
                            skip_group_check=True,
                        )

                for kb in range(nkb):
                    dg = kb - 4 * qb  # diagonal block index (0..3) if >= 0
                    lo = 128 * dg if dg > 0 else 0  # first live q column
                    # S^T duo: both heads of the pair for this key block
                    qd = psq.tile([128, 2, 512], F32, tag="qd")
                    for h in range(2):
                        nc.tensor.matmul(
                            qd[:, h, lo:512],
                            kt[64 * h : 64 * (h + 1), pr, kb * 128 : (kb + 1) * 128],
                            qt[64 * h : 64 * (h + 1), pr, qb * 512 + lo : (qb + 1) * 512],
                            start=True,
                            stop=True,
                        )
                    pt_t = ptp.tile([128, 2, 512], BF16, tag="pt")
                    nc.scalar.activation(
                        pt_t[:, :, lo:512], qd[:, :, lo:512], EXP, scale=SCALE
                    )
                    if dg >= 0:
                        # only the 128-wide triangle band needs the 0/1 mask
                        sl = slice(128 * dg, 128 * dg + 128)
                        nc.vector.tensor_mul(pt_t[:, :, sl], pt_t[:, :, sl], mk_sb[:])
                    emit_pv(kb, pt_t, lo)
                    if extra and kb in extra:
                        extra[kb]()
                    if dg < 0:
                        drip()
                if last:
                    # final block: no next attention needs the PSUM slots, so
                    # normalize straight from PSUM (no bounce).  1/den runs as
                    # exp(-ln(den)) on the now-idle Scalar queue (same act
                    # table as the softmax exp), so out-proj pr=1 unblocks
                    # ~3us sooner than the 3.3us DVE reciprocal would allow.
                    LN = mybir.ActivationFunctionType.Ln
                    for h in range(2):
                        lnt = rcp1.tile([1, 512], F32, tag=f"lnt{h}", name=f"lnt_{h}")
                        nc.scalar.activation(lnt[:], o_ps[h][64:65, :], LN)
                        rec = rcp1.tile([1, 512], F32, tag=f"lrec{h}", name=f"lrec_{h}")
                        nc.scalar.activation(rec[:], lnt[:], EXP, scale=-1.0)
                        br = rcp1.tile([64, 512], F32, tag=f"brs{h}", name=f"lbrs_{h}")
                        nc.gpsimd.partition_broadcast(br[:], rec[:])
                        for par in range(2):
                            nc.vector.tensor_mul(
                                o2t_all[pr][h][
                                    64 * par : 64 * par + 64,
                                    256 * qb : 256 * (qb + 1),
                                ],
                                o_ps[h][0:64, par::2],
                                br[:, par::2],
                            )
                    return
                # Bounce O~ + denominator rows to SBUF now: frees the PSUM
                # slots; the recip/broadcast/mult tail drips into later idle
                # vector slots (see make_normalize_tail).
                osb_h = []
                den2 = rcp.tile([33, 512], F32, tag="den2", name=f"den2_{pr}_{qb}")
                nc.gpsimd.memset(den2[:], 1.0)
                for h in range(2):
                    osb = osbp.tile(
                        [64, 512], F32, tag="osb", name=f"osb_{pr}_{qb}_{h}"
                    )
                    nc.vector.tensor_copy(osb[:], o_ps[h][0:64, :])
                    nc.vector.tensor_copy(den2[32 * h : 32 * h + 1, :], o_ps[h][64:65, :])
                    osb_h.append(osb)
                tails[0].extend(make_normalize_tail(pr, qb, osb_h, den2))

            def outproj_group(pr, h, oh, copy_eng=None):
                hl = 2 * pr + h
                op = pss.tile([128, 512], F32, tag="sc")
                for m in range(8):
                    nc.tensor.matmul(
                        op[:],
                        o2t_all[pr][h][:, m::8],
                        wo_parts[m // 2][:, m % 2, oh * 512 : (oh + 1) * 512],
                        start=(m == 0),
                        stop=(m == 7),
                    )
                osb = obp.tile([128, 512], F32, tag="ob")
                if copy_eng is nc.vector:
                    nc.vector.tensor_copy(osb[:], op[:])
                else:
                    nc.scalar.copy(osb[:], op[:])
                nc.sync.dma_start(
                    out_d[hl * 128 : (hl + 1) * 128, oh * 512 : (oh + 1) * 512],
                    osb[:],
                )

            OP_REST = [(0, 1, 1), (1, 0, 0), (1, 0, 1), (1, 1, 0), (1, 1, 1)]

            # --- fused pipeline: chunk c+1's Q/K/V projection runs
            #     between attention(0,c) and attention(1,c), so the new
            #     qt/kt/vt (and their scalar copies) are ready a full block
            #     before attention(0,c+1) needs them, and projection matmuls
            #     fill any PE stalls in the attention tail ---
            def load_chunk(c):
                xc = xp.tile([128, NDB, 512], BF16, tag="xc", name=f"xc_{c}")
                nc.sync.dma_start(xc[:, 0:4, :], xt_r[:, 0:4, c * 512 : (c + 1) * 512])
                nc.sync.dma_start(xc[:, 4:8, :], xt_r[:, 4:8, c * 512 : (c + 1) * 512])
                return xc

            def project(c, xc):
                for w_sb, dst in ((wq_sb, qt), (wk_sb, kt)):
                    for mg in range(2):
                        ps = pss.tile([128, 512], F32, tag="sc")
                        for do in range(NDB):
                            nc.tensor.matmul(
                                ps[:],
                                w_sb[:, do, mg * 128 : (mg + 1) * 128],
                                xc[:, do, :],
                                start=(do == 0),
                                stop=(do == NDB - 1),
                            )
                        nc.scalar.copy(
                            dst[:, mg, c * 512 : (c + 1) * 512], ps[:]
                        )
                        drip()
                for kbl in range(4):
                    kb = 4 * c + kbl
                    ps = pss.tile([128, 512], F32, tag="sc")
                    nps = ps[:, 0:256]
                    for do in range(NDB):
                        nc.tensor.matmul(
                            nps,
                            xc[:, do, kbl * 128 : (kbl + 1) * 128],
                            wv_sb[:, do, :],
                            start=(do == 0),
                            stop=(do == NDB - 1),
                        )
                    nc.scalar.copy(
                        vt[:, kb, :, 0:64], nps.rearrange("p (h e) -> p h e", h=HPC)
                    )
                    drip()

            project(0, xc0)
            for c in range(NQB):
                attention(0, qb=c)
                if c + 1 < NQB:
                    project(c + 1, load_chunk(c + 1))
                else:
                    # Wo m=4..7 issue while attention(0,3) still runs
                    for i, tg in ((2, "wo2c"), (3, "wo2d")):
                        wpart = wp.tile([128, 2, D], F32R, tag=tg, name=f"wo_{i}")
                        nc.sync.dma_start(wpart[:], wo_d[:, 2 * i : 2 * i + 2, :])
                        wo_parts.append(wpart)
                if c == NQB - 1:
                    attention(
                        1,
                        qb=c,
                        last=True,
                        extra={
                            6: lambda: outproj_group(0, 0, 0, nc.vector),
                            9: lambda: outproj_group(0, 0, 1, nc.vector),
                            12: lambda: outproj_group(0, 1, 0, nc.vector),
                        },
                    )
                else:
                    attention(1, qb=c)

            # remaining out-projection groups (pr0 h1 oh1 + all of pr1);
            # pr0's first three ran interleaved inside attention(1,3)
            for pr, h, oh in OP_REST:
                outproj_group(pr, h, oh)
                drip(2)
            drip(16)

    nc.compile()
    return nc


def _get_nc():
    if "nc" not in _CACHE:
        _CACHE["nc"] = build_nc()
    return _CACHE["nc"]


def _prep_w(wg):
    """(4, 64, 1024) per-head weights -> [128, 8, 256] SBUF lhsT layout."""
    # WT[d, f=(h*64+e)] = wg[h, e, d]; block d = do*128 + p -> [p, do, f]
    wt = wg.transpose(2, 0, 1).reshape(D, 256)
    return np.ascontiguousarray(
        wt.reshape(NDB, 128, 256).transpose(1, 0, 2)
    ).astype(BF16_NP)


def _prep_wo(wot):
    """WoT (1024, 1024) [c, o] -> [128, 8, 1024] with c = 128*m + p."""
    return np.ascontiguousarray(wot.reshape(8, 128, D).transpose(1, 0, 2))


def make_in_maps(x, Wq_lb, Wk_lb, Wv_lb, Wq_la, Wk_la, Wv_la, Wo):
    B = x.shape[0]
    xf = np.asarray(x, np.float32).reshape(B, N, D)
    wot = np.ascontiguousarray(np.asarray(Wo, np.float32).T)  # [c, o]
    wot_rev = np.ascontiguousarray(wot.reshape(16, 64, D)[::-1].reshape(D, D))
    wo_maps = {False: _prep_wo(wot), True: _prep_wo(wot_rev)}

    kp = np.arange(128, dtype=np.int64)[:, None]
    jf = np.arange(128, dtype=np.int64)[None, :]
    tri = (jf >= kp).astype(BF16_NP)  # [128, 128] causal triangle
    mk = np.ascontiguousarray(np.stack([tri, tri], axis=1))  # [128, 2, 128]

    xts = {}
    for b in range(B):
        xts[(b, False)] = np.ascontiguousarray(xf[b].T).astype(BF16_NP)
        xts[(b, True)] = np.ascontiguousarray(xf[b][::-1].T).astype(BF16_NP)

    wsel = {
        False: (np.asarray(Wq_lb, np.float32), np.asarray(Wk_lb, np.float32),
                np.asarray(Wv_lb, np.float32)),
        True: (np.asarray(Wq_la, np.float32), np.asarray(Wk_la, np.float32),
               np.asarray(Wv_la, np.float32)),
    }
    wcache = {}
    in_maps = []
    for c in range(8):
        b, grp = divmod(c, 4)
        la = grp >= 2
        half = grp % 2
        key = (la, half)
        if key not in wcache:
            wq, wk, wv = wsel[la]
            sl = slice(half * 4, half * 4 + 4)
            wcache[key] = (_prep_w(wq[sl]), _prep_w(wk[sl]), _prep_w(wv[sl]))
        pwq, pwk, pwv = wcache[key]
        in_maps.append(
            {
                "xt": xts[(b, la)],
                "wq": pwq,
                "wk": pwk,
                "wv": pwv,
                "wo": wo_maps[la],
                "mk": mk,
            }
        )
    return in_maps


def assemble(results, B=2):
    out = np.empty((B, N, D), np.float32)
    for c in range(8):
        b, grp = divmod(c, 4)
        part = np.asarray(results[c]["out"])  # (512, 1024)
        if grp >= 2:  # lookahead: un-reverse rows within each head block
            part = part.reshape(HPC, 128, D)[:, ::-1].reshape(512, D)
        out[b, grp * 512 : (grp + 1) * 512] = part
    return out


def kernel(x, Wq_lb, Wk_lb, Wv_lb, Wq_la, Wk_la, Wv_la, Wo):
    nc = _get_nc()
    in_maps = make_in_maps(x, Wq_lb, Wk_lb, Wv_lb, Wq_la, Wk_la, Wv_la, Wo)
    res = run_bass_kernel_spmd(nc, in_maps, list(range(8)))
    B, T, F_, D_ = x.shape
    return assemble(res.results, B).reshape(B, T, F_, D_)
